# revision 22
# baseline (speedup 1.0000x reference)
"""Multi-head attention with full attn_bias, sharded over 8 TRN2 NeuronCores.

Reference math (B=4, N=2048, C=768, H=12, D=64):
    q,k,v = heads(x @ W{q,k,v}.T);  S = q k^T * D^-0.5 + bias
    out = softmax(S) v;  y = merge(out) @ Wp.T + bp

Sharding: 8 cores = 4 head-groups (3 heads) x 2 query-row halves (1024 rows).
Each core computes, for its 3 heads: K/V over all tokens (all 4 batches) and
Q over its 1024 rows, then scores TRANSPOSED S^T[k, q] so softmax's sum runs
along the PSUM free dim of the AV matmul.  The attn bias is folded into the
score accumulation with an identity matmul (PSUM accumulate), exp runs on
ScalarE with no max-subtraction (logits here are ~N(0, sqrt(2)); exp cannot
overflow fp32), and the softmax denominator comes free from a ones column
appended to V.  Per-core partial projections (contraction over 192 of 768
features) are summed on the host, which also adds bp.
"""

import os

import numpy as np

import concourse.bass as bass
from concourse import bacc
import concourse.mybir as mybir
import concourse.tile as tile
from concourse.bass_utils import run_bass_kernel_spmd

B, N, C, H, D = 4, 2048, 768, 12, 64
SCALE = D ** -0.5
HG = 3            # heads per core
FH = HG * D       # 192 features per core
QH = N // 2       # 1024 q rows per core
KC = N // 128     # 16 key chunks
CC = C // 128     # 6 contraction chunks
F32 = mybir.dt.float32
F32R = mybir.dt.float32r
Exp = mybir.ActivationFunctionType.Exp

_cache = {}


def build_nc():
    nc = bacc.Bacc(None, target_bir_lowering=False)
    xT = nc.dram_tensor("xT", [B, C, N], F32R, kind="ExternalInput")
    wqT = nc.dram_tensor("wqT", [C, FH], F32R, kind="ExternalInput")
    wkT = nc.dram_tensor("wkT", [C, FH], F32R, kind="ExternalInput")
    wvT = nc.dram_tensor("wvT", [C, 256], F32R, kind="ExternalInput")
    wpT = nc.dram_tensor("wpT", [FH, C], F32R, kind="ExternalInput")
    biasT = nc.dram_tensor("biasT", [HG, KC, 128, QH], F32R, kind="ExternalInput")
    ident = nc.dram_tensor("ident", [128, 128], F32R, kind="ExternalInput")
    on128 = nc.dram_tensor("on128", [128, 4], F32R, kind="ExternalInput")
    yT = nc.dram_tensor("yT", [B, C, QH], F32, kind="ExternalOutput")
    # V spilled to DRAM between phases; layout per (b, kc): 128 tokens x
    # [h0 d0..63, 1, h1 d0..63, 1, h2 d0..63, 1] so each head's (V | ones)
    # block is a contiguous 65-column slice.
    vs = nc.dram_tensor("vscratch", [B, KC, 128, 195], F32R, kind="Internal")

    with tile.TileContext(nc) as tc:
        with (
            nc.allow_low_precision(reason="fp32r data; all PSUM accum is fp32"),
            tc.tile_pool(name="singles", bufs=1) as singles,
            tc.tile_pool(name="qk", bufs=1) as qk,
            tc.tile_pool(name="stream", bufs=6) as stream,
            tc.tile_pool(name="vstage", bufs=3) as vstage,
            tc.tile_pool(name="small", bufs=4) as small,
            tc.tile_pool(name="ostore", bufs=1) as ostore,
            tc.tile_pool(name="ystage", bufs=3) as ypool,
            tc.tile_pool(name="ps", bufs=3, space="PSUM") as ps,
            tc.tile_pool(name="psav", bufs=4, space="PSUM") as psav,
            tc.tile_pool(name="pst", bufs=1, space="PSUM") as pst,
        ):
            # ---- phase 0: weights ----
            wq_s = singles.tile([128, CC, FH], F32R)
            wk_s = singles.tile([128, CC, FH], F32R)
            wv_s = singles.tile([128, CC, 256], F32R)
            nc.sync.dma_start(out=wq_s, in_=wqT.rearrange("(c p) m -> p c m", p=128))
            nc.sync.dma_start(out=wk_s, in_=wkT.rearrange("(c p) m -> p c m", p=128))
            nc.sync.dma_start(out=wv_s, in_=wvT.rearrange("(c p) m -> p c m", p=128))
            wp_s = singles.tile([64, HG, C], F32R)
            for fc in range(HG):
                nc.sync.dma_start(out=wp_s[0:64, fc, :],
                                  in_=wpT[64 * fc:64 * fc + 64, :])
            id_s = singles.tile([128, 128], F32R)
            nc.sync.dma_start(out=id_s, in_=ident[:, :])
            ones_s = singles.tile([1, 64], F32)
            nc.vector.memset(ones_s, 1.0)
            on_s = singles.tile([128, 4], F32R)
            nc.sync.dma_start(out=on_s, in_=on128[:, :])
            trash = pst.tile([2, 8], F32)

            def touch(ap):
                # fp32r matmuls get a single codegen wait slot; a 1x1 dummy
                # matmul absorbs a fresh DMA/engine dependency into the PE
                # vector clock so the real matmul needs at most one wait.
                idx = tuple(slice(0, 1) for _ in ap.shape[:-1]) + (slice(0, 2),)
                one = ap[idx]
                nc.tensor.matmul(trash[0:2, 0:2], one, one,
                                 start=True, stop=True, skip_group_check=True)

            for w in (wq_s, wk_s, wv_s, id_s, wp_s):
                touch(w)

            # Persistent per-batch tensors. h2 (the 64-wide tail of the 192
            # features) is packed batch-pair-wise into full 128-partition tiles.
            qtA = [qk.tile([128, QH], F32R, name=f"qtA{b}") for b in range(B)]
            qtB = [qk.tile([128, QH], F32R, name=f"qtB{p}") for p in range(B // 2)]
            ktA = [qk.tile([128, N], F32R, name=f"ktA{b}") for b in range(B)]
            ktB = [qk.tile([128, N], F32R, name=f"ktB{p}") for p in range(B // 2)]
            # O^T store: all (h, qt) slots at base partition 0 — fp32r
            # accumulation groups with base-64 operands crash the HW.
            ot = [ostore.tile([64, 2 * HG, 512], F32R, name=f"ot{b}")
                  for b in range(B)]

            def q_slice(b, h, qt):
                if h < 2:
                    return qtA[b][64 * h:64 * h + 64, qt * 512:qt * 512 + 512]
                return qtB[b // 2][64 * (b % 2):64 * (b % 2) + 64,
                                   qt * 512:qt * 512 + 512]

            def k_slice(b, h, kc):
                if h < 2:
                    return ktA[b][64 * h:64 * h + 64, kc * 128:kc * 128 + 128]
                return ktB[b // 2][64 * (b % 2):64 * (b % 2) + 64,
                                   kc * 128:kc * 128 + 128]

            def o_slice(b, h, qt):
                return ot[b][0:64, 2 * h + qt, :]

            # ---- phase 1: QKV projections, V spilled to DRAM ----
            # Processed per 1024-token half so the shared stream slots stay
            # at [128, 1024].
            for b in range(B):
              for th in range(2):
                xc = [stream.tile([128, QH], F32R, tag="stream",
                                  name=f"xc{b}_{th}_{c}") for c in range(CC)]
                for c in range(CC):
                    nc.sync.dma_start(
                        out=xc[c],
                        in_=xT[b, c * 128:c * 128 + 128,
                               th * QH:th * QH + QH])
                    touch(xc[c])
                # Q^T [192, 1024] (rows 0..1023 are this core's q tokens)
                for qt in range(2 if th == 0 else 0):
                    pq = ps.tile([128, 512], F32, tag="ps", name=f"pq{b}{qt}")
                    for c in range(CC):
                        nc.tensor.matmul(pq, wq_s[:, c, 0:128],
                                         xc[c][:, qt * 512:qt * 512 + 512],
                                         start=(c == 0), stop=(c == CC - 1))
                    nc.vector.tensor_copy(qtA[b][:, qt * 512:qt * 512 + 512], pq)
                    pq2 = ps.tile([128, 512], F32, tag="ps", name=f"pq2{b}{qt}")
                    for c in range(CC):
                        nc.tensor.matmul(pq2[0:64, :], wq_s[:, c, 128:192],
                                         xc[c][:, qt * 512:qt * 512 + 512],
                                         start=(c == 0), stop=(c == CC - 1))
                    nc.vector.tensor_copy(
                        qtB[b // 2][64 * (b % 2):64 * (b % 2) + 64,
                                    qt * 512:qt * 512 + 512], pq2[0:64, :])
                # K^T [192, 2048]
                for tl in range(2):
                    t = th * 2 + tl
                    pk = ps.tile([128, 512], F32, tag="ps", name=f"pk{b}{t}")
                    for c in range(CC):
                        nc.tensor.matmul(pk, wk_s[:, c, 0:128],
                                         xc[c][:, tl * 512:tl * 512 + 512],
                                         start=(c == 0), stop=(c == CC - 1))
                    nc.vector.tensor_copy(ktA[b][:, t * 512:t * 512 + 512], pk)
                    pk2 = ps.tile([128, 512], F32, tag="ps", name=f"pk2{b}{t}")
                    for c in range(CC):
                        nc.tensor.matmul(pk2[0:64, :], wk_s[:, c, 128:192],
                                         xc[c][:, tl * 512:tl * 512 + 512],
                                         start=(c == 0), stop=(c == CC - 1))
                    nc.vector.tensor_copy(
                        ktB[b // 2][64 * (b % 2):64 * (b % 2) + 64,
                                    t * 512:t * 512 + 512], pk2[0:64, :])
                # V [2048 tokens, 192] directly token-major (N padded to 256
                # to stay at full fp32r rate), then spill per 128-token chunk.
                for ktl in range(KC // 2):
                    kt = th * (KC // 2) + ktl
                    pv = ps.tile([128, 256], F32, tag="ps", name=f"pv{b}{kt}")
                    for c in range(CC):
                        nc.tensor.matmul(pv, xc[c][:, ktl * 128:ktl * 128 + 128],
                                         wv_s[:, c, :],
                                         start=(c == 0), stop=(c == CC - 1))
                    vst = vstage.tile([128, 195], F32R, tag="vstage",
                                      name=f"vst{b}{kt}")
                    nc.vector.tensor_copy(
                        bass.AP(tensor=vst.tensor, offset=vst.offset,
                                ap=[list(vst.ap[0]), [65, 3], [1, 64]]),
                        bass.AP(tensor=pv.tensor, offset=pv.offset,
                                ap=[list(pv.ap[0]), [64, 3], [1, 64]]))
                    nc.vector.tensor_copy(
                        bass.AP(tensor=vst.tensor, offset=vst.offset + 64,
                                ap=[list(vst.ap[0]), [65, 3]]),
                        on_s[:, 0:3])
                    nc.sync.dma_start(out=vs[b, kt], in_=vst)

            PH = int(os.environ.get("KPHASES", "3"))
            # ---- phase 2: scores + softmax + AV, bias streamed once ----
            for h in range(HG if PH >= 2 else 0):
                for qt in range(2):
                    av = [psav.tile([128, 512], F32, tag="av", name=f"av{h}{qt}{b}")
                          for b in range(B)]
                    for kc in range(KC):
                        bt = stream.tile([128, 512], F32R, tag="stream",
                                         name=f"bt{h}{qt}{kc}")
                        nc.sync.dma_start(
                            out=bt, in_=biasT[h, kc, :, qt * 512:qt * 512 + 512])
                        touch(bt)
                        vt4 = vstage.tile([128, B, 65], F32R, tag="vt",
                                          name=f"vt{h}{qt}{kc}")
                        nc.sync.dma_start(
                            out=vt4,
                            in_=vs[:, kc, :, 65 * h:65 * h + 65].rearrange(
                                "b p c -> p b c"))
                        touch(vt4)
                        for b in range(B):
                            sp = ps.tile([128, 512], F32, tag="ps",
                                         name=f"sp{h}{qt}{kc}{b}")
                            nc.tensor.matmul(sp, id_s, bt, start=True, stop=False)
                            nc.tensor.matmul(sp, k_slice(b, h, kc),
                                             q_slice(b, h, qt),
                                             start=False, stop=True)
                            pt = stream.tile([128, 512], F32R, tag="stream",
                                             name=f"pt{h}{qt}{kc}{b}")
                            nc.scalar.activation(pt, sp, Exp)
                            if kc == 0:
                                touch(pt)
                            nc.tensor.matmul(av[b][0:65, :], vt4[:, b, :], pt,
                                             start=(kc == 0), stop=(kc == KC - 1))
                    for b in range(B):
                        rec = small.tile([1, 512], F32, tag="rec",
                                         name=f"rec{h}{qt}{b}")
                        nc.vector.reciprocal(rec, av[b][64:65, :])
                        touch(rec)
                        bc_ps = ps.tile([64, 512], F32, tag="ps",
                                        name=f"bcp{h}{qt}{b}")
                        nc.tensor.matmul(bc_ps, ones_s, rec,
                                         start=True, stop=True)
                        bc = small.tile([64, 512], F32, tag="bc",
                                        name=f"bc{h}{qt}{b}")
                        nc.scalar.copy(bc, bc_ps)
                        nc.vector.tensor_mul(o_slice(b, h, qt),
                                             av[b][0:64, :], bc)

            # ---- phase 3: output projection (partial over 192 features) ----
            for b in range(B):
                if PH < 3:
                    yst0 = ypool.tile([128, 512], F32, tag="y", name=f"yz{b}")
                    nc.vector.tensor_copy(yst0, qtA[b][:, 0:512])
                    nc.sync.dma_start(out=yT[b, 0:128, 0:512], in_=yst0)
                    continue
                for oc in range(CC):
                    for qt in range(2):
                        py = ps.tile([128, 512], F32, tag="ps",
                                     name=f"py{b}{oc}{qt}")
                        for fc in range(HG):
                            lhs = wp_s[0:64, fc, oc * 128:oc * 128 + 128]
                            nc.tensor.matmul(py, lhs, o_slice(b, fc, qt),
                                             start=(fc == 0), stop=(fc == HG - 1))
                        yst = ypool.tile([128, 512], F32, tag="y",
                                         name=f"y{b}{oc}{qt}")
                        nc.vector.tensor_copy(yst, py)
                        nc.sync.dma_start(
                            out=yT[b, oc * 128:oc * 128 + 128,
                                   qt * 512:qt * 512 + 512], in_=yst)
    nc.finalize()
    return nc


def kernel(x, attn_bias, Wq, Wk, Wv, Wp, bp):
    x = np.asarray(x, np.float32)
    attn_bias = np.asarray(attn_bias, np.float32)
    Wq, Wk, Wv, Wp, bp = (np.asarray(a, np.float32) for a in (Wq, Wk, Wv, Wp, bp))
    if "nc" not in _cache:
        _cache["nc"] = build_nc()
    nc = _cache["nc"]

    ident = np.eye(128, dtype=np.float32)
    in_maps = []
    for core in range(8):
        hg, qh = core // 2, core % 2
        hr = slice(hg * FH, (hg + 1) * FH)
        perm = np.r_[qh * QH:(qh + 1) * QH, (1 - qh) * QH:(1 - qh) * QH + QH]
        xp = np.ascontiguousarray(x[:, perm, :].transpose(0, 2, 1))
        wq = np.ascontiguousarray((Wq[hr] * SCALE).T)
        wk = np.ascontiguousarray(Wk[hr].T)
        wv = np.concatenate(
            [Wv[hr].T, np.zeros((C, 256 - FH), np.float32)], axis=1)
        ones128 = np.ones((128, 4), np.float32)
        wp = np.ascontiguousarray(Wp[:, hr].T)
        bt = np.ascontiguousarray(
            attn_bias[0, hg * HG:(hg + 1) * HG][:, qh * QH:(qh + 1) * QH][:, :, perm]
            .transpose(0, 2, 1)).reshape(HG, KC, 128, QH)
        in_maps.append(dict(xT=xp, wqT=wq, wkT=wk, wvT=np.ascontiguousarray(wv),
                            wpT=wp, biasT=bt, ident=ident, on128=ones128))

    import time as _time
    t0 = _time.perf_counter()
    res = run_bass_kernel_spmd(nc, in_maps, core_ids=list(range(8)))
    kernel.last_exec_s = _time.perf_counter() - t0
    y = np.zeros((B, N, C), np.float32)
    for core in range(8):
        qh = core % 2
        y[:, qh * QH:(qh + 1) * QH, :] += res.results[core]["yT"].transpose(0, 2, 1)
    return y + bp


# revision 24
# speedup vs baseline: 1.0641x; 1.0641x over previous
"""Multi-head attention with full attn_bias, sharded over 8 TRN2 NeuronCores.

Reference math (B=4, N=2048, C=768, H=12, D=64):
    q,k,v = heads(x @ W{q,k,v}.T);  S = q k^T * D^-0.5 + bias
    out = softmax(S) v;  y = merge(out) @ Wp.T + bp

Sharding: 8 cores = 4 head-groups (3 heads) x 2 query-row halves (1024 rows).
Each core computes, for its 3 heads: K/V over all tokens (all 4 batches) and
Q over its 1024 rows, then scores TRANSPOSED S^T[k, q] so softmax's sum runs
along the PSUM free dim of the AV matmul.  The attn bias is folded into the
score accumulation with an identity matmul (PSUM accumulate), exp runs on
ScalarE with no max-subtraction (logits here are ~N(0, sqrt(2)); exp cannot
overflow fp32), and the softmax denominator comes free from a ones column
appended to V.  Per-core partial projections (contraction over 192 of 768
features) are summed on the host, which also adds bp.
"""

import os

import numpy as np

import concourse.bass as bass
from concourse import bacc
import concourse.mybir as mybir
import concourse.tile as tile
from concourse.bass_utils import run_bass_kernel_spmd

B, N, C, H, D = 4, 2048, 768, 12, 64
SCALE = D ** -0.5
HG = 3            # heads per core
FH = HG * D       # 192 features per core
QH = N // 2       # 1024 q rows per core
KC = N // 128     # 16 key chunks
CC = C // 128     # 6 contraction chunks
F32 = mybir.dt.float32
F32R = mybir.dt.float32r
Exp = mybir.ActivationFunctionType.Exp

_cache = {}


def build_nc():
    nc = bacc.Bacc(None, target_bir_lowering=False)
    xT = nc.dram_tensor("xT", [B, C, N], F32R, kind="ExternalInput")
    wqT = nc.dram_tensor("wqT", [C, FH], F32R, kind="ExternalInput")
    wkT = nc.dram_tensor("wkT", [C, FH], F32R, kind="ExternalInput")
    wvT = nc.dram_tensor("wvT", [C, 256], F32R, kind="ExternalInput")
    wpT = nc.dram_tensor("wpT", [FH, C], F32R, kind="ExternalInput")
    biasT = nc.dram_tensor("biasT", [HG, KC, 128, QH], F32R, kind="ExternalInput")
    ident = nc.dram_tensor("ident", [128, 128], F32R, kind="ExternalInput")
    on128 = nc.dram_tensor("on128", [128, 4], F32R, kind="ExternalInput")
    yT = nc.dram_tensor("yT", [B, C, QH], F32, kind="ExternalOutput")
    # V spilled to DRAM between phases; layout per (b, kc): 128 tokens x
    # [h0 d0..63, 1, h1 d0..63, 1, h2 d0..63, 1] so each head's (V | ones)
    # block is a contiguous 65-column slice.
    vs = nc.dram_tensor("vscratch", [B, KC, 128, 195], F32R, kind="Internal")

    with tile.TileContext(nc) as tc:
        with (
            nc.allow_low_precision(reason="fp32r data; all PSUM accum is fp32"),
            tc.tile_pool(name="singles", bufs=1) as singles,
            tc.tile_pool(name="qk", bufs=1) as qk,
            tc.tile_pool(name="stream", bufs=6) as stream,
            tc.tile_pool(name="vstage", bufs=3) as vstage,
            tc.tile_pool(name="small", bufs=4) as small,
            tc.tile_pool(name="ostore", bufs=1) as ostore,
            tc.tile_pool(name="ystage", bufs=3) as ypool,
            tc.tile_pool(name="ps", bufs=4, space="PSUM") as ps,
            tc.tile_pool(name="psav", bufs=4, space="PSUM") as psav,
        ):
            # ---- phase 0: weights ----
            wq_s = singles.tile([128, CC, FH], F32R)
            wk_s = singles.tile([128, CC, FH], F32R)
            wv_s = singles.tile([128, CC, 256], F32R)
            nc.sync.dma_start(out=wq_s, in_=wqT.rearrange("(c p) m -> p c m", p=128))
            nc.sync.dma_start(out=wk_s, in_=wkT.rearrange("(c p) m -> p c m", p=128))
            nc.sync.dma_start(out=wv_s, in_=wvT.rearrange("(c p) m -> p c m", p=128))
            wp_s = singles.tile([64, HG, C], F32R)
            for fc in range(HG):
                nc.sync.dma_start(out=wp_s[0:64, fc, :],
                                  in_=wpT[64 * fc:64 * fc + 64, :])
            id_s = singles.tile([128, 128], F32R)
            nc.sync.dma_start(out=id_s, in_=ident[:, :])
            ones_s = singles.tile([1, 64], F32)
            nc.vector.memset(ones_s, 1.0)
            on_s = singles.tile([128, 4], F32R)
            nc.sync.dma_start(out=on_s, in_=on128[:, :])

            # Persistent per-batch tensors. h2 (the 64-wide tail of the 192
            # features) is packed batch-pair-wise into full 128-partition tiles.
            qtA = [qk.tile([128, QH], F32R, name=f"qtA{b}") for b in range(B)]
            qtB = [qk.tile([128, QH], F32R, name=f"qtB{p}") for p in range(B // 2)]
            ktA = [qk.tile([128, N], F32R, name=f"ktA{b}") for b in range(B)]
            ktB = [qk.tile([128, N], F32R, name=f"ktB{p}") for p in range(B // 2)]
            # O^T store: all (h, qt) slots at base partition 0 — fp32r
            # accumulation groups with base-64 operands crash the HW.
            ot = [ostore.tile([64, 2 * HG, 512], F32R, name=f"ot{b}")
                  for b in range(B)]

            def q_slice(b, h, qt):
                if h < 2:
                    return qtA[b][64 * h:64 * h + 64, qt * 512:qt * 512 + 512]
                return qtB[b // 2][64 * (b % 2):64 * (b % 2) + 64,
                                   qt * 512:qt * 512 + 512]

            def k_slice(b, h, kc):
                if h < 2:
                    return ktA[b][64 * h:64 * h + 64, kc * 128:kc * 128 + 128]
                return ktB[b // 2][64 * (b % 2):64 * (b % 2) + 64,
                                   kc * 128:kc * 128 + 128]

            def o_slice(b, h, qt):
                return ot[b][0:64, 2 * h + qt, :]

            # ---- phase 1: QKV projections, V spilled to DRAM ----
            # Processed per 1024-token half so the shared stream slots stay
            # at [128, 1024].
            for b in range(B):
              for th in range(2):
                xc = [stream.tile([128, QH], F32R, tag="stream",
                                  name=f"xc{b}_{th}_{c}") for c in range(CC)]
                for c in range(CC):
                    nc.sync.dma_start(
                        out=xc[c],
                        in_=xT[b, c * 128:c * 128 + 128,
                               th * QH:th * QH + QH])
                # Q^T [192, 1024] (rows 0..1023 are this core's q tokens)
                for qt in range(2 if th == 0 else 0):
                    pq = ps.tile([128, 512], F32, tag="ps", name=f"pq{b}{qt}")
                    for c in range(CC):
                        nc.tensor.matmul(pq, wq_s[:, c, 0:128],
                                         xc[c][:, qt * 512:qt * 512 + 512],
                                         start=(c == 0), stop=(c == CC - 1))
                    nc.vector.tensor_copy(qtA[b][:, qt * 512:qt * 512 + 512], pq)
                    pq2 = ps.tile([128, 512], F32, tag="ps", name=f"pq2{b}{qt}")
                    for c in range(CC):
                        nc.tensor.matmul(pq2[0:64, :], wq_s[:, c, 128:192],
                                         xc[c][:, qt * 512:qt * 512 + 512],
                                         start=(c == 0), stop=(c == CC - 1))
                    nc.vector.tensor_copy(
                        qtB[b // 2][64 * (b % 2):64 * (b % 2) + 64,
                                    qt * 512:qt * 512 + 512], pq2[0:64, :])
                # K^T [192, 2048]
                for tl in range(2):
                    t = th * 2 + tl
                    pk = ps.tile([128, 512], F32, tag="ps", name=f"pk{b}{t}")
                    for c in range(CC):
                        nc.tensor.matmul(pk, wk_s[:, c, 0:128],
                                         xc[c][:, tl * 512:tl * 512 + 512],
                                         start=(c == 0), stop=(c == CC - 1))
                    nc.vector.tensor_copy(ktA[b][:, t * 512:t * 512 + 512], pk)
                    pk2 = ps.tile([128, 512], F32, tag="ps", name=f"pk2{b}{t}")
                    for c in range(CC):
                        nc.tensor.matmul(pk2[0:64, :], wk_s[:, c, 128:192],
                                         xc[c][:, tl * 512:tl * 512 + 512],
                                         start=(c == 0), stop=(c == CC - 1))
                    nc.vector.tensor_copy(
                        ktB[b // 2][64 * (b % 2):64 * (b % 2) + 64,
                                    t * 512:t * 512 + 512], pk2[0:64, :])
                # V [2048 tokens, 192] directly token-major (N padded to 256
                # to stay at full fp32r rate), then spill per 128-token chunk.
                for ktl in range(KC // 2):
                    kt = th * (KC // 2) + ktl
                    pv = ps.tile([128, 256], F32, tag="ps", name=f"pv{b}{kt}")
                    for c in range(CC):
                        nc.tensor.matmul(pv, xc[c][:, ktl * 128:ktl * 128 + 128],
                                         wv_s[:, c, :],
                                         start=(c == 0), stop=(c == CC - 1))
                    vst = vstage.tile([128, 195], F32R, tag="vstage",
                                      name=f"vst{b}{kt}")
                    nc.vector.tensor_copy(
                        bass.AP(tensor=vst.tensor, offset=vst.offset,
                                ap=[list(vst.ap[0]), [65, 3], [1, 64]]),
                        bass.AP(tensor=pv.tensor, offset=pv.offset,
                                ap=[list(pv.ap[0]), [64, 3], [1, 64]]))
                    nc.vector.tensor_copy(
                        bass.AP(tensor=vst.tensor, offset=vst.offset + 64,
                                ap=[list(vst.ap[0]), [65, 3]]),
                        on_s[:, 0:3])
                    nc.sync.dma_start(out=vs[b, kt], in_=vst)

            PH = int(os.environ.get("KPHASES", "3"))
            # ---- phase 2: scores + softmax + AV, bias streamed once ----
            for h in range(HG if PH >= 2 else 0):
                for qt in range(2):
                    av = [psav.tile([128, 512], F32, tag="av", name=f"av{h}{qt}{b}")
                          for b in range(B)]
                    for kc in range(KC):
                        bt = stream.tile([128, 512], F32R, tag="stream",
                                         name=f"bt{h}{qt}{kc}")
                        nc.sync.dma_start(
                            out=bt, in_=biasT[h, kc, :, qt * 512:qt * 512 + 512])
                        vt4 = vstage.tile([128, B, 65], F32R, tag="vt",
                                          name=f"vt{h}{qt}{kc}")
                        nc.sync.dma_start(
                            out=vt4,
                            in_=vs[:, kc, :, 65 * h:65 * h + 65].rearrange(
                                "b p c -> p b c"))
                        for b in range(B):
                            sp = ps.tile([128, 512], F32, tag="ps",
                                         name=f"sp{h}{qt}{kc}{b}")
                            nc.tensor.matmul(sp, id_s, bt, start=True, stop=False)
                            nc.tensor.matmul(sp, k_slice(b, h, kc),
                                             q_slice(b, h, qt),
                                             start=False, stop=True)
                            pt = stream.tile([128, 512], F32R, tag="stream",
                                             name=f"pt{h}{qt}{kc}{b}")
                            nc.scalar.activation(pt, sp, Exp)
                            nc.tensor.matmul(av[b][0:65, :], vt4[:, b, :], pt,
                                             start=(kc == 0), stop=(kc == KC - 1))
                    for b in range(B):
                        rec = small.tile([1, 512], F32, tag="rec",
                                         name=f"rec{h}{qt}{b}")
                        nc.vector.reciprocal(rec, av[b][64:65, :])
                        bc_ps = ps.tile([64, 512], F32, tag="ps",
                                        name=f"bcp{h}{qt}{b}")
                        nc.tensor.matmul(bc_ps, ones_s, rec,
                                         start=True, stop=True)
                        bc = small.tile([64, 512], F32, tag="bc",
                                        name=f"bc{h}{qt}{b}")
                        nc.scalar.copy(bc, bc_ps)
                        nc.vector.tensor_mul(o_slice(b, h, qt),
                                             av[b][0:64, :], bc)

            # ---- phase 3: output projection (partial over 192 features) ----
            for b in range(B):
                if PH < 3:
                    yst0 = ypool.tile([128, 512], F32, tag="y", name=f"yz{b}")
                    nc.vector.tensor_copy(yst0, qtA[b][:, 0:512])
                    nc.sync.dma_start(out=yT[b, 0:128, 0:512], in_=yst0)
                    continue
                for oc in range(CC):
                    for qt in range(2):
                        py = ps.tile([128, 512], F32, tag="ps",
                                     name=f"py{b}{oc}{qt}")
                        for fc in range(HG):
                            lhs = wp_s[0:64, fc, oc * 128:oc * 128 + 128]
                            nc.tensor.matmul(py, lhs, o_slice(b, fc, qt),
                                             start=(fc == 0), stop=(fc == HG - 1))
                        yst = ypool.tile([128, 512], F32, tag="y",
                                         name=f"y{b}{oc}{qt}")
                        nc.vector.tensor_copy(yst, py)
                        nc.sync.dma_start(
                            out=yT[b, oc * 128:oc * 128 + 128,
                                   qt * 512:qt * 512 + 512], in_=yst)
    nc.finalize()
    return nc


def kernel(x, attn_bias, Wq, Wk, Wv, Wp, bp):
    x = np.asarray(x, np.float32)
    attn_bias = np.asarray(attn_bias, np.float32)
    Wq, Wk, Wv, Wp, bp = (np.asarray(a, np.float32) for a in (Wq, Wk, Wv, Wp, bp))
    if "nc" not in _cache:
        _cache["nc"] = build_nc()
    nc = _cache["nc"]

    ident = np.eye(128, dtype=np.float32)
    in_maps = []
    for core in range(8):
        hg, qh = core // 2, core % 2
        hr = slice(hg * FH, (hg + 1) * FH)
        perm = np.r_[qh * QH:(qh + 1) * QH, (1 - qh) * QH:(1 - qh) * QH + QH]
        xp = np.ascontiguousarray(x[:, perm, :].transpose(0, 2, 1))
        wq = np.ascontiguousarray((Wq[hr] * SCALE).T)
        wk = np.ascontiguousarray(Wk[hr].T)
        wv = np.concatenate(
            [Wv[hr].T, np.zeros((C, 256 - FH), np.float32)], axis=1)
        ones128 = np.ones((128, 4), np.float32)
        wp = np.ascontiguousarray(Wp[:, hr].T)
        bt = np.ascontiguousarray(
            attn_bias[0, hg * HG:(hg + 1) * HG][:, qh * QH:(qh + 1) * QH][:, :, perm]
            .transpose(0, 2, 1)).reshape(HG, KC, 128, QH)
        in_maps.append(dict(xT=xp, wqT=wq, wkT=wk, wvT=np.ascontiguousarray(wv),
                            wpT=wp, biasT=bt, ident=ident, on128=ones128))

    import time as _time
    t0 = _time.perf_counter()
    res = run_bass_kernel_spmd(nc, in_maps, core_ids=list(range(8)))
    kernel.last_exec_s = _time.perf_counter() - t0
    y = np.zeros((B, N, C), np.float32)
    for core in range(8):
        qh = core % 2
        y[:, qh * QH:(qh + 1) * QH, :] += res.results[core]["yT"].transpose(0, 2, 1)
    return y + bp
